# revision 15
# baseline (speedup 1.0000x reference)
"""Attention-decoder (B=128, T=256, F=512, O=512, MID=1000, 32 steps) on 8 trn2 cores.

Data-parallel over batch (16 per core), split into two 8-batch half-pipelines
per step so one half's softmax/LSTM tail hides under the other half's tanh
block (the Tile scheduler overlaps them automatically).

Per half-step:
  u = W1s @ s                       (PE, 32 matmuls free-8 into one [128,64] PSUM)
  x = pre + broadcast_t(u)          (DVE tensor_tensor 2x, 8 ops of free 2048)
  h = tanh(x + b1)                  (ACT, 8 ops of free 2048, per-partition bias)
  logits = W2 @ h                   (PE, 8 matmuls free 2048, b-major cols, fp16 PSUM)
  PSUM->SBUF logits copy            (Pool/GPSIMD -- otherwise idle)
  reshape [1,2048]->[8,256]         (DMA, 8 descriptors)
  alphas = softmax(relu(l+b2))      (ACT exp + DVE, exp(relu(x)) == max(exp(x),1))
  alphT [t,b]                       (PE transpose free-8)
  ctx.T [f,b]                       (PE, 64 rank-1 matmuls of free 1)
  gates [o,b]                       (PE, 144 matmuls free-8; sigmoid halving folded
                                     into weights so all 4 gates are plain tanh)
  LSTM elementwise on [128,(oc,b)] combined tiles; s stays [o,b] fp16 so the
  recurrence needs no transposes.  Output [wo,128,(h,oc,b)] fp16, untangled on host.
"""
import sys
import numpy as np

sys.path.insert(0, "/opt/trn_rl_repo")

B, T, F, O, MID = 128, 256, 512, 512, 1000
MIDP = 1024  # padded
NCORES = 8
BC = B // NCORES  # 16 batch per core
BH = BC // 2      # 8 per half-pipeline
BT = BC * T       # 4096
BTH = BH * T      # 2048


def _build(wo: int):
    import concourse.bass as bass
    import concourse.bacc as bacc
    import concourse.mybir as mybir
    from concourse.tile import TileContext

    f16 = mybir.dt.float16
    f32 = mybir.dt.float32
    AF = mybir.ActivationFunctionType
    OP = mybir.AluOpType
    AX = mybir.AxisListType

    nc = bacc.Bacc()
    aT_d = nc.dram_tensor("aT", [F, BT], f16, kind="ExternalInput")       # t-major cols
    aN_d = nc.dram_tensor("aN", [BT, F], f16, kind="ExternalInput")       # b-major rows
    W1aT_d = nc.dram_tensor("W1aT", [F, MIDP], f16, kind="ExternalInput")
    W1sT_d = nc.dram_tensor("W1sT", [O, MIDP], f16, kind="ExternalInput")
    W2c_d = nc.dram_tensor("W2c", [128, 8], f16, kind="ExternalInput")
    b1T_d = nc.dram_tensor("b1T", [128, 8], f32, kind="ExternalInput")
    b2_d = nc.dram_tensor("b2s", [BC, 1], f32, kind="ExternalInput")
    WgT_d = nc.dram_tensor("WgT", [O + F, 4 * O], f16, kind="ExternalInput")
    bgr_d = nc.dram_tensor("bgr", [1, 4 * O], f16, kind="ExternalInput")
    sP_d = nc.dram_tensor("sP", [128, 4 * BC], f16, kind="ExternalInput")  # [p,(h,oc,bh)]
    eyeh_d = nc.dram_tensor("eyeh", [16, 16], f16, kind="ExternalInput")
    ones_d = nc.dram_tensor("ones16", [1, BC], f16, kind="ExternalInput")
    out_d = nc.dram_tensor("out", [wo, 128, 4 * BC], f16, kind="ExternalOutput")

    with TileContext(nc) as tc:
        with (
            tc.tile_pool(name="const", bufs=1) as cp,
            tc.tile_pool(name="xh", bufs=2) as xp,
            tc.tile_pool(name="small", bufs=2) as sp,
            tc.tile_pool(name="state", bufs=2) as wp,
            tc.tile_pool(name="astream", bufs=2) as app,
            tc.tile_pool(name="psbig", bufs=2, space="PSUM") as psbig,
            tc.tile_pool(name="pslog", bufs=1, space="PSUM") as pslog,
            tc.tile_pool(name="pscr", bufs=2, space="PSUM") as pscr,
        ):
            dma = nc.sync.dma_start

            # ---- constant loads ----
            aN_sb = {}
            for b in range(BC):
                for tcn in range(2):
                    t_ = cp.tile([128, F], f16, tag=f"aN{b}_{tcn}", name=f"aN{b}_{tcn}")
                    dma(t_[:], aN_d[b * T + tcn * 128: b * T + (tcn + 1) * 128, :])
                    aN_sb[(b, tcn)] = t_
            W1aT_sb, W1sT_sb, WgT_sb = [], [], []
            for kc in range(4):
                t_ = cp.tile([128, MIDP], f16, tag=f"w1a{kc}", name=f"w1a{kc}")
                dma(t_[:], W1aT_d[kc * 128:(kc + 1) * 128, :])
                W1aT_sb.append(t_)
            for kc in range(4):
                t_ = cp.tile([128, MIDP], f16, tag=f"w1s{kc}", name=f"w1s{kc}")
                dma(t_[:], W1sT_d[kc * 128:(kc + 1) * 128, :])
                W1sT_sb.append(t_)
            for kc in range(8):
                t_ = cp.tile([128, 4 * O], f16, tag=f"wg{kc}", name=f"wg{kc}")
                dma(t_[:], WgT_d[kc * 128:(kc + 1) * 128, :])
                WgT_sb.append(t_)
            W2_sb = cp.tile([128, 8], f16, tag="w2", name="w2")
            dma(W2_sb[:], W2c_d[:])
            b1T_sb = cp.tile([128, 8], f32, tag="b1t", name="b1t")
            dma(b1T_sb[:], b1T_d[:])
            b2_sb = cp.tile([BC, 1], f32, tag="b2", name="b2")
            dma(b2_sb[:], b2_d[:])
            bgr_sb = cp.tile([1, 4 * O], f16, tag="bgr", name="bgr")
            dma(bgr_sb[:], bgr_d[:])
            eyeh_sb = cp.tile([16, 16], f16, tag="eyeh", name="eyeh")
            dma(eyeh_sb[:], eyeh_d[:])
            ones_sb = cp.tile([1, BC], f16, tag="ones", name="ones")
            dma(ones_sb[:], ones_d[:])

            s_cur = [None, None]
            for h in range(2):
                t_ = wp.tile([128, 4 * BH], f16, tag=f"s{h}", name=f"s{h}_0")
                dma(t_[:], sP_d[:, h * 4 * BH:(h + 1) * 4 * BH])
                s_cur[h] = t_
            c_cur = [None, None]
            for h in range(2):
                t_ = wp.tile([128, 4 * BH], f32, tag=f"c{h}", name=f"c{h}_0")
                nc.vector.memset(t_[:], 0.0)
                c_cur[h] = t_

            # ---- precompute pre = (a @ W1a.T).T : [MID_p, (t,b)] fp16 ----
            pre_sb = []
            for mc in range(8):
                pre_sb.append(cp.tile([128, BT], f16, tag=f"pre{mc}", name=f"pre{mc}"))
            for ns in range(8):
                a_sl = []
                for kc in range(4):
                    t_ = app.tile([128, 512], f16, tag=f"astr{kc}", name=f"astr{kc}")
                    dma(t_[:], aT_d[kc * 128:(kc + 1) * 128, ns * 512:(ns + 1) * 512])
                    a_sl.append(t_)
                for mc in range(8):
                    ps = psbig.tile([128, 512], f32, tag="psbig", name="psbig")
                    for kc in range(4):
                        nc.tensor.matmul(
                            ps[:],
                            W1aT_sb[kc][:, mc * 128:(mc + 1) * 128],
                            a_sl[kc][:],
                            start=(kc == 0), stop=(kc == 3),
                        )
                    dst = pre_sb[mc][:, ns * 512:(ns + 1) * 512]
                    if mc % 2 == 0:
                        nc.scalar.copy(dst, ps[:])
                    else:
                        nc.vector.tensor_copy(dst, ps[:])

            # ---- decode steps, two half-pipelines ----
            for t in range(wo):
                for h in range(2):
                    s_h = s_cur[h]
                    sT = [s_h[:, kc * BH:(kc + 1) * BH] for kc in range(4)]

                    # one 2KB psum scratch bank per half-step: psu | pst | psc | gates
                    scr = pscr.tile([128, 240], f32, tag="scr", name="scr")
                    psu = scr[:, 0:64]
                    pst_s = [scr[:, 64:68].bitcast(f16), scr[:, 68:72].bitcast(f16)]
                    psc_s = [scr[:, 72 + fc * 8:80 + fc * 8] for fc in range(4)]
                    pgc = scr[:, 104:136]
                    pgs = scr[:, 136:232]
                    for mc in range(8):
                        for kc in range(4):
                            nc.tensor.matmul(
                                psu[:, mc * BH:(mc + 1) * BH],
                                W1sT_sb[kc][:, mc * 128:(mc + 1) * 128],
                                sT[kc],
                                start=(kc == 0), stop=(kc == 3),
                            )
                    u_sb = sp.tile([128, 8 * BH], f16, tag=f"u{h}", name=f"u{h}")
                    with tc.high_priority(offset=700):
                        nc.vector.tensor_copy(u_sb[:], psu)

                    pl = pslog.tile([1, BTH], f32, tag="pl", name=f"pl{h}")
                    for mc in range(8):
                        x_ = xp.tile([128, BTH], f16, tag=f"x{h}", name=f"x{h}")
                        x3 = x_[:].rearrange("p (t b) -> p t b", t=T, b=BH)
                        pre3 = pre_sb[mc][:].rearrange(
                            "p (t b) -> p t b", t=T, b=BC)[:, :, h * BH:(h + 1) * BH]
                        ub = u_sb[:, mc * BH:(mc + 1) * BH].unsqueeze(1).broadcast_to(
                            (128, T, BH))
                        with tc.high_priority(offset=700):
                            nc.vector.tensor_tensor(out=x3, in0=pre3, in1=ub, op=OP.add)
                        h_ = xp.tile([128, BTH], f16, tag=f"h{h}", name=f"h{h}")
                        nc.scalar.activation(
                            h_[:], x_[:], AF.Tanh,
                            bias=b1T_sb[:, mc:mc + 1], scale=1.0,
                        )
                        for q in range(4):
                            nc.tensor.matmul(
                                pl[0:1, q * 512:(q + 1) * 512],
                                W2_sb[:, mc:mc + 1],
                                h_[:, q * 512:(q + 1) * 512],
                                start=(mc == 0), stop=(mc == 7),
                            )

                    # logits -> SBUF (DVE; GPSIMD cannot read PSUM) -> [8,256] DMA
                    lrow = sp.tile([1, BTH], f16, tag=f"lrow{h}", name=f"lrow{h}", bufs=1)
                    lrow_v = lrow[:].rearrange("p (b t) -> p t b", b=BH, t=T)
                    pl_v = pl[:].rearrange("p (t b) -> p t b", t=T, b=BH)
                    for q in range(4):
                        nc.vector.tensor_copy(lrow_v[:, q * 64:(q + 1) * 64, :],
                                              pl_v[:, q * 64:(q + 1) * 64, :])
                    lq = sp.tile([BH, T], f16, tag=f"lq{h}", name=f"lq{h}", bufs=1)
                    dma(lq[:], lrow[0:1, :])

                    # softmax over t: alphas = exp(relu(l+b2))/sum
                    esb = sp.tile([BH, T], f16, tag=f"esb{h}", name=f"esb{h}", bufs=1)
                    nc.scalar.activation(esb[:], lq[:], AF.Exp,
                                         bias=b2_sb[0:BH, 0:1], scale=1.0)
                    esc = sp.tile([BH, T], f16, tag=f"esc{h}", name=f"esc{h}", bufs=1)
                    nc.vector.tensor_scalar(out=esc[:], in0=esb[:], scalar1=1.0,
                                            scalar2=None, op0=OP.max)
                    ssum = sp.tile([BH, 1], f32, tag=f"ssum{h}", name=f"ssum{h}")
                    nc.vector.tensor_reduce(ssum[:], esc[:], AX.X, OP.add)
                    inv = sp.tile([BH, 1], f32, tag=f"inv{h}", name=f"inv{h}")
                    nc.vector.reciprocal(inv[:], ssum[:])
                    alph = sp.tile([BH, T], f16, tag=f"alph{h}", name=f"alph{h}", bufs=1)
                    nc.vector.tensor_scalar(out=alph[:], in0=esc[:],
                                            scalar1=inv[:, 0:1], scalar2=None,
                                            op0=OP.mult)
                    alphT = []
                    for tcn in range(2):
                        pst = pst_s[tcn]
                        nc.tensor.transpose(pst, alph[:, tcn * 128:(tcn + 1) * 128],
                                            eyeh_sb[0:BH, 0:BH])
                        at_ = sp.tile([128, BH], f16, tag=f"alphT{tcn}_{h}",
                                      name=f"alphT{tcn}_{h}")
                        nc.vector.tensor_copy(at_[:], pst)
                        alphT.append(at_)

                    # ctx.T [f, b]: rank-1 matmuls (free size 1)
                    ctxT = []
                    for fc in range(4):
                        psc = psc_s[fc]
                        for b in range(BH):
                            gb = h * BH + b
                            for tcn in range(2):
                                nc.tensor.matmul(
                                    psc[:, b:b + 1],
                                    aN_sb[(gb, tcn)][:, fc * 128:(fc + 1) * 128],
                                    alphT[tcn][:, b:b + 1],
                                    start=(tcn == 0), stop=(tcn == 1),
                                )
                        ct_ = sp.tile([128, BH], f16, tag=f"ctxT{fc}_{h}",
                                      name=f"ctxT{fc}_{h}")
                        nc.vector.tensor_copy(ct_[:], psc)
                        ctxT.append(ct_)

                    # gates in [o, b] layout; cand separate (full-scale tanh)
                    for g in range(4):
                        for oc in range(4):
                            if g == 0:
                                dst = pgc[:, oc * BH:(oc + 1) * BH]
                            else:
                                i = (g - 1) * 4 + oc
                                dst = pgs[:, i * BH:(i + 1) * BH]
                            col = g * O + oc * 128
                            nc.tensor.matmul(dst, bgr_sb[0:1, col:col + 128],
                                             ones_sb[0:1, 0:BH], start=True, stop=False)
                            for kc in range(4):
                                nc.tensor.matmul(
                                    dst, WgT_sb[kc][:, col:col + 128], sT[kc],
                                    start=False, stop=False,
                                )
                            for kc in range(4):
                                nc.tensor.matmul(
                                    dst, WgT_sb[4 + kc][:, col:col + 128], ctxT[kc][:],
                                    start=False, stop=(kc == 3),
                                )
                    cand = sp.tile([128, 4 * BH], f16, tag=f"cand{h}", name=f"cand{h}")
                    nc.scalar.activation(cand[:], pgc, AF.Tanh)
                    sig_t = sp.tile([128, 12 * BH], f16, tag=f"sigt{h}", name=f"sigt{h}")
                    nc.scalar.activation(sig_t[:], pgs, AF.Tanh)
                    sig = sp.tile([128, 12 * BH], f16, tag=f"sig{h}", name=f"sig{h}")
                    nc.vector.tensor_scalar(out=sig[:], in0=sig_t[:], scalar1=0.5,
                                            scalar2=0.5, op0=OP.mult, op1=OP.add)

                    t1 = sp.tile([128, 4 * BH], f32, tag=f"t1{h}", name=f"t1{h}")
                    nc.vector.tensor_tensor(out=t1[:], in0=sig[:, 0:4 * BH],
                                            in1=cand[:], op=OP.mult)
                    t2 = sp.tile([128, 4 * BH], f32, tag=f"t2{h}", name=f"t2{h}")
                    nc.vector.tensor_tensor(out=t2[:], in0=sig[:, 4 * BH:8 * BH],
                                            in1=c_cur[h][:], op=OP.mult)
                    c_new = wp.tile([128, 4 * BH], f32, tag=f"c{h}", name=f"c{h}")
                    nc.vector.tensor_tensor(out=c_new[:], in0=t1[:], in1=t2[:], op=OP.add)
                    tch = sp.tile([128, 4 * BH], f16, tag=f"tch{h}", name=f"tch{h}")
                    nc.scalar.activation(tch[:], c_new[:], AF.Tanh)
                    s_new = wp.tile([128, 4 * BH], f16, tag=f"s{h}", name=f"s{h}")
                    nc.vector.tensor_tensor(out=s_new[:], in0=sig[:, 8 * BH:12 * BH],
                                            in1=tch[:], op=OP.mult)

                    dma(out_d[t, :, h * 4 * BH:(h + 1) * 4 * BH], s_new[:])
                    c_cur[h] = c_new
                    s_cur[h] = s_new
    nc.compile()
    return nc


def _make_runner(nc):
    """Build the sharded jit callable ONCE per module (run_bass_via_pjrt
    rebuilds it per call, costing seconds of retrace/recompile)."""
    import jax
    import numpy as _np
    from jax.sharding import Mesh, PartitionSpec
    from jax.experimental.shard_map import shard_map
    from concourse import bass2jax, mybir

    bass2jax.install_neuronx_cc_hook()
    partition_name = nc.partition_id_tensor.name if nc.partition_id_tensor else None
    in_names, out_names, out_avals, zero_outs = [], [], [], []
    for alloc in nc.m.functions[0].allocations:
        if not isinstance(alloc, mybir.MemoryLocationSet):
            continue
        name = alloc.memorylocations[0].name
        if alloc.kind == "ExternalInput":
            if name != partition_name:
                in_names.append(name)
        elif alloc.kind == "ExternalOutput":
            shape = tuple(alloc.tensor_shape)
            dtype = mybir.dt.np(alloc.dtype)
            out_names.append(name)
            out_avals.append(jax.core.ShapedArray(shape, dtype))
            zero_outs.append(_np.zeros(shape, dtype))
    n_params = len(in_names)
    n_outs = len(out_avals)
    in_names_all = list(in_names) + list(out_names)
    if partition_name is not None:
        in_names_all.append(partition_name)

    def _body(*args):
        operands = list(args)
        if partition_name is not None:
            operands.append(bass2jax.partition_id_tensor())
        outs = bass2jax._bass_exec_p.bind(
            *operands,
            out_avals=tuple(out_avals),
            in_names=tuple(in_names_all),
            out_names=tuple(out_names),
            lowering_input_output_aliases=(),
            sim_require_finite=True,
            sim_require_nnan=True,
            nc=nc,
        )
        return tuple(outs)

    donate = tuple(range(n_params, n_params + n_outs))
    devices = jax.devices()[:NCORES]
    mesh = Mesh(_np.asarray(devices), ("core",))
    sharded = jax.jit(
        shard_map(_body, mesh=mesh,
                  in_specs=(PartitionSpec("core"),) * (n_params + n_outs),
                  out_specs=(PartitionSpec("core"),) * n_outs,
                  check_rep=False),
        donate_argnums=donate, keep_unused=True,
    )

    def run(in_maps):
        concat_in = [
            np.concatenate([np.asarray(in_maps[c][nm]) for c in range(NCORES)], axis=0)
            for nm in in_names[:n_params]
        ]
        concat_zeros = [np.zeros((NCORES * z.shape[0], *z.shape[1:]), z.dtype)
                        for z in zero_outs]
        out_arrs = sharded(*concat_in, *concat_zeros)
        return [
            {nm: np.asarray(out_arrs[i]).reshape(NCORES, *out_avals[i].shape)[c]
             for i, nm in enumerate(out_names)}
            for c in range(NCORES)
        ]

    run.sharded = sharded
    run.zero_outs = zero_outs
    run.in_names = in_names[:n_params]
    run.out_names = out_names
    run.out_avals = out_avals
    return run


_BUILT = {}


def kernel(**inputs):
    a = np.asarray(inputs["a"], np.float32)
    s_prev = np.asarray(inputs["s_prev"], np.float32)
    W1 = np.asarray(inputs["W1"], np.float32)
    b1 = np.asarray(inputs["b1"], np.float32)
    W2 = np.asarray(inputs["W2"], np.float32)
    b2 = np.asarray(inputs["b2"], np.float32)
    w_c = np.asarray(inputs["w_c"], np.float32)
    w_u = np.asarray(inputs["w_u"], np.float32)
    w_f = np.asarray(inputs["w_f"], np.float32)
    w_o = np.asarray(inputs["w_o"], np.float32)
    b_c = np.asarray(inputs["b_c"], np.float32)
    b_u = np.asarray(inputs["b_u"], np.float32)
    b_f = np.asarray(inputs["b_f"], np.float32)
    b_o = np.asarray(inputs["b_o"], np.float32)
    wo = int(np.asarray(inputs["word_output"]))

    if wo not in _BUILT:
        nc_ = _build(wo)
        _BUILT[wo] = (nc_, _make_runner(nc_))
    nc, runner = _BUILT[wo]

    W1aT = np.zeros((F, MIDP), np.float16)
    W1aT[:, :MID] = W1[:, :F].T
    W1sT = np.zeros((O, MIDP), np.float16)
    W1sT[:, :MID] = W1[:, F:].T
    W2p = np.zeros((MIDP,), np.float32)
    W2p[:MID] = W2[0]
    W2c = W2p.reshape(8, 128).T.astype(np.float16)
    b1p = np.zeros((MIDP,), np.float32)
    b1p[:MID] = b1
    b1T = b1p.reshape(8, 128).T.copy()
    # gate weights: candidate full scale; u/f/o gates pre-scaled by 0.5 so
    # sigmoid(z) = 0.5*tanh(z/2)+0.5 becomes plain tanh on-device.
    WgT = np.concatenate(
        [w_c.T] + [0.5 * w.T for w in (w_u, w_f, w_o)], axis=1).astype(np.float16)
    bgr = np.concatenate(
        [b_c] + [0.5 * b for b in (b_u, b_f, b_o)]).reshape(1, 4 * O).astype(np.float16)
    common = {
        "W1aT": W1aT, "W1sT": W1sT, "W2c": W2c, "b1T": b1T,
        "b2s": np.full((BC, 1), float(b2.reshape(-1)[0]), np.float32),
        "WgT": WgT, "bgr": bgr,
        "eyeh": np.eye(16, dtype=np.float16),
        "ones16": np.ones((1, BC), np.float16),
    }
    in_maps = []
    for c in range(NCORES):
        b0 = c * BC
        ac = a[b0:b0 + BC]
        # aT t-major: aT[f, t*BC + b] = a[b, t, f]
        aT = np.ascontiguousarray(ac.transpose(2, 1, 0).reshape(F, BT)).astype(np.float16)
        # sP [p, (h,oc,bh)]: sP[p, h*32+oc*8+bh] = s_prev[b0 + h*8+bh, oc*128+p]
        sP = np.ascontiguousarray(
            np.transpose(s_prev[b0:b0 + BC].T.reshape(4, 128, 2, BH), (1, 2, 0, 3))
        ).reshape(128, 4 * BC).astype(np.float16)
        in_maps.append({
            **common,
            "aT": aT,
            "aN": np.ascontiguousarray(ac.reshape(BT, F)).astype(np.float16),
            "sP": sP,
        })

    results = None
    for attempt in range(4):
        try:
            results = runner(in_maps)
            break
        except Exception:
            if attempt == 3:
                raise
            import time as _time
            _time.sleep(1.0)
            if attempt >= 1:
                runner = _make_runner(nc)
                _BUILT[wo] = (nc, runner)
    out = np.empty((B, wo, O), np.float32)
    for c in range(NCORES):
        # raw [wo, 128, (h,oc,bh)] fp16 -> [b = h*8+bh, wo, o = oc*128+p]
        raw = results[c]["out"].reshape(wo, 128, 2, 4, BH)
        out[c * BC:(c + 1) * BC] = np.ascontiguousarray(
            np.transpose(raw, (2, 4, 0, 3, 1))).reshape(BC, wo, O).astype(np.float32)
    return out


# revision 16
# speedup vs baseline: 1.1438x; 1.1438x over previous
"""Attention-decoder (B=128, T=256, F=512, O=512, MID=1000, 32 steps) on 8 trn2 cores.

Data-parallel over batch (16 per core), split into two 8-batch half-pipelines
per step so one half's softmax/LSTM tail hides under the other half's tanh
block (the Tile scheduler overlaps them automatically).

Per half-step:
  u = W1s @ s                       (PE, 32 matmuls free-8 into one [128,64] PSUM)
  x = pre + broadcast_t(u)          (DVE tensor_tensor 2x, 8 ops of free 2048)
  h = tanh(x + b1)                  (ACT, 8 ops of free 2048, per-partition bias)
  logits = W2 @ h                   (PE, 8 matmuls free 2048, b-major cols, fp16 PSUM)
  PSUM->SBUF logits copy            (Pool/GPSIMD -- otherwise idle)
  reshape [1,2048]->[8,256]         (DMA, 8 descriptors)
  alphas = softmax(relu(l+b2))      (ACT exp + DVE, exp(relu(x)) == max(exp(x),1))
  alphT [t,b]                       (PE transpose free-8)
  ctx.T [f,b]                       (PE, 64 rank-1 matmuls of free 1)
  gates [o,b]                       (PE, 144 matmuls free-8; sigmoid halving folded
                                     into weights so all 4 gates are plain tanh)
  LSTM elementwise on [128,(oc,b)] combined tiles; s stays [o,b] fp16 so the
  recurrence needs no transposes.  Output [wo,128,(h,oc,b)] fp16, untangled on host.
"""
import sys
import numpy as np

sys.path.insert(0, "/opt/trn_rl_repo")

B, T, F, O, MID = 128, 256, 512, 512, 1000
MIDP = 1024  # padded
NCORES = 8
BC = B // NCORES  # 16 batch per core
BH = BC // 2      # 8 per half-pipeline
BT = BC * T       # 4096
BTH = BH * T      # 2048


def _build(wo: int):
    import concourse.bass as bass
    import concourse.bacc as bacc
    import concourse.mybir as mybir
    from concourse.tile import TileContext

    f16 = mybir.dt.float16
    f32 = mybir.dt.float32
    AF = mybir.ActivationFunctionType
    OP = mybir.AluOpType
    AX = mybir.AxisListType

    nc = bacc.Bacc()
    aT_d = nc.dram_tensor("aT", [F, BT], f16, kind="ExternalInput")       # t-major cols
    aN_d = nc.dram_tensor("aN", [BT, F], f16, kind="ExternalInput")       # b-major rows
    W1aT_d = nc.dram_tensor("W1aT", [F, MIDP], f16, kind="ExternalInput")
    W1sT_d = nc.dram_tensor("W1sT", [O, MIDP], f16, kind="ExternalInput")
    W2c_d = nc.dram_tensor("W2c", [128, 8], f16, kind="ExternalInput")
    b1T_d = nc.dram_tensor("b1T", [128, 8], f32, kind="ExternalInput")
    b2_d = nc.dram_tensor("b2s", [BC, 1], f32, kind="ExternalInput")
    WgT_d = nc.dram_tensor("WgT", [O + F, 4 * O], f16, kind="ExternalInput")
    bgr_d = nc.dram_tensor("bgr", [1, 4 * O], f16, kind="ExternalInput")
    sP_d = nc.dram_tensor("sP", [128, 4 * BC], f16, kind="ExternalInput")  # [p,(h,oc,bh)]
    eyeh_d = nc.dram_tensor("eyeh", [16, 16], f16, kind="ExternalInput")
    ones_d = nc.dram_tensor("ones16", [1, BC], f16, kind="ExternalInput")
    out_d = nc.dram_tensor("out", [wo, 128, 4 * BC], f16, kind="ExternalOutput")

    with TileContext(nc) as tc:
        with (
            tc.tile_pool(name="const", bufs=1) as cp,
            tc.tile_pool(name="xh", bufs=2) as xp,
            tc.tile_pool(name="small", bufs=2) as sp,
            tc.tile_pool(name="state", bufs=2) as wp,
            tc.tile_pool(name="astream", bufs=2) as app,
            tc.tile_pool(name="psbig", bufs=2, space="PSUM") as psbig,
            tc.tile_pool(name="pslog", bufs=1, space="PSUM") as pslog,
            tc.tile_pool(name="pscr", bufs=2, space="PSUM") as pscr,
        ):
            dma = nc.sync.dma_start

            # ---- constant loads ----
            aN_sb = {}
            for b in range(BC):
                for tcn in range(2):
                    t_ = cp.tile([128, F], f16, tag=f"aN{b}_{tcn}", name=f"aN{b}_{tcn}")
                    dma(t_[:], aN_d[b * T + tcn * 128: b * T + (tcn + 1) * 128, :])
                    aN_sb[(b, tcn)] = t_
            W1aT_sb, W1sT_sb, WgT_sb = [], [], []
            for kc in range(4):
                t_ = cp.tile([128, MIDP], f16, tag=f"w1a{kc}", name=f"w1a{kc}")
                dma(t_[:], W1aT_d[kc * 128:(kc + 1) * 128, :])
                W1aT_sb.append(t_)
            for kc in range(4):
                t_ = cp.tile([128, MIDP], f16, tag=f"w1s{kc}", name=f"w1s{kc}")
                dma(t_[:], W1sT_d[kc * 128:(kc + 1) * 128, :])
                W1sT_sb.append(t_)
            for kc in range(8):
                t_ = cp.tile([128, 4 * O], f16, tag=f"wg{kc}", name=f"wg{kc}")
                dma(t_[:], WgT_d[kc * 128:(kc + 1) * 128, :])
                WgT_sb.append(t_)
            W2_sb = cp.tile([128, 8], f16, tag="w2", name="w2")
            dma(W2_sb[:], W2c_d[:])
            b1T_sb = cp.tile([128, 8], f32, tag="b1t", name="b1t")
            dma(b1T_sb[:], b1T_d[:])
            b2_sb = cp.tile([BC, 1], f32, tag="b2", name="b2")
            dma(b2_sb[:], b2_d[:])
            bgr_sb = cp.tile([1, 4 * O], f16, tag="bgr", name="bgr")
            dma(bgr_sb[:], bgr_d[:])
            eyeh_sb = cp.tile([16, 16], f16, tag="eyeh", name="eyeh")
            dma(eyeh_sb[:], eyeh_d[:])
            ones_sb = cp.tile([1, BC], f16, tag="ones", name="ones")
            dma(ones_sb[:], ones_d[:])

            s_cur = [None, None]
            for h in range(2):
                t_ = wp.tile([128, 4 * BH], f16, tag=f"s{h}", name=f"s{h}_0")
                dma(t_[:], sP_d[:, h * 4 * BH:(h + 1) * 4 * BH])
                s_cur[h] = t_
            c_cur = [None, None]
            for h in range(2):
                t_ = wp.tile([128, 4 * BH], f32, tag=f"c{h}", name=f"c{h}_0")
                nc.vector.memset(t_[:], 0.0)
                c_cur[h] = t_

            # ---- precompute pre = (a @ W1a.T).T : [MID_p, (t,b)] fp16 ----
            pre_sb = []
            for mc in range(8):
                pre_sb.append(cp.tile([128, BT], f16, tag=f"pre{mc}", name=f"pre{mc}"))
            for ns in range(8):
                a_sl = []
                for kc in range(4):
                    t_ = app.tile([128, 512], f16, tag=f"astr{kc}", name=f"astr{kc}")
                    dma(t_[:], aT_d[kc * 128:(kc + 1) * 128, ns * 512:(ns + 1) * 512])
                    a_sl.append(t_)
                for mc in range(8):
                    ps = psbig.tile([128, 512], f32, tag="psbig", name="psbig")
                    for kc in range(4):
                        nc.tensor.matmul(
                            ps[:],
                            W1aT_sb[kc][:, mc * 128:(mc + 1) * 128],
                            a_sl[kc][:],
                            start=(kc == 0), stop=(kc == 3),
                        )
                    dst = pre_sb[mc][:, ns * 512:(ns + 1) * 512]
                    if mc % 2 == 0:
                        nc.scalar.copy(dst, ps[:])
                    else:
                        nc.vector.tensor_copy(dst, ps[:])

            # ---- decode steps, two half-pipelines ----
            for t in range(wo):
                for h in range(2):
                    s_h = s_cur[h]
                    sT = [s_h[:, kc * BH:(kc + 1) * BH] for kc in range(4)]

                    # one 2KB psum scratch bank per half-step: psu | pst | psc | gates
                    scr = pscr.tile([128, 240], f32, tag="scr", name="scr")
                    psu = scr[:, 0:64]
                    pst_s = [scr[:, 64:68].bitcast(f16), scr[:, 68:72].bitcast(f16)]
                    psc_s = [scr[:, 72 + fc * 8:80 + fc * 8] for fc in range(4)]
                    pgc = scr[:, 104:136]
                    pgs = scr[:, 136:232]
                    for mc in range(8):
                        for kc in range(4):
                            nc.tensor.matmul(
                                psu[:, mc * BH:(mc + 1) * BH],
                                W1sT_sb[kc][:, mc * 128:(mc + 1) * 128],
                                sT[kc],
                                start=(kc == 0), stop=(kc == 3),
                            )
                    u_sb = sp.tile([128, 8 * BH], f16, tag=f"u{h}", name=f"u{h}")
                    with tc.high_priority(offset=700):
                        nc.vector.tensor_copy(u_sb[:], psu)

                    pl = pslog.tile([1, BTH], f32, tag="pl", name=f"pl{h}")
                    for mc in range(8):
                        x_ = xp.tile([128, BTH], f16, tag=f"x{h}", name=f"x{h}")
                        x3 = x_[:].rearrange("p (t b) -> p t b", t=T, b=BH)
                        pre3 = pre_sb[mc][:].rearrange(
                            "p (t b) -> p t b", t=T, b=BC)[:, :, h * BH:(h + 1) * BH]
                        ub = u_sb[:, mc * BH:(mc + 1) * BH].unsqueeze(1).broadcast_to(
                            (128, T, BH))
                        with tc.high_priority(offset=700):
                            nc.vector.tensor_tensor(out=x3, in0=pre3, in1=ub, op=OP.add)
                        h_ = xp.tile([128, BTH], f16, tag=f"h{h}", name=f"h{h}")
                        nc.scalar.activation(
                            h_[:], x_[:], AF.Tanh,
                            bias=b1T_sb[:, mc:mc + 1], scale=1.0,
                        )
                        for q in range(4):
                            nc.tensor.matmul(
                                pl[0:1, q * 512:(q + 1) * 512],
                                W2_sb[:, mc:mc + 1],
                                h_[:, q * 512:(q + 1) * 512],
                                start=(mc == 0), stop=(mc == 7),
                            )

                    # logits -> SBUF (DVE; GPSIMD cannot read PSUM) -> [8,256] DMA
                    lrow = sp.tile([1, BTH], f16, tag=f"lrow{h}", name=f"lrow{h}", bufs=1)
                    nc.vector.tensor_copy(
                        lrow[:].rearrange("p (b t) -> p t b", b=BH, t=T),
                        pl[:].rearrange("p (t b) -> p t b", t=T, b=BH))
                    lq = sp.tile([BH, T], f16, tag=f"lq{h}", name=f"lq{h}", bufs=1)
                    dma(lq[:], lrow[0:1, :])

                    # softmax over t: alphas = exp(relu(l+b2))/sum
                    esb = sp.tile([BH, T], f16, tag=f"esb{h}", name=f"esb{h}", bufs=1)
                    nc.scalar.activation(esb[:], lq[:], AF.Exp,
                                         bias=b2_sb[0:BH, 0:1], scale=1.0)
                    esc = sp.tile([BH, T], f16, tag=f"esc{h}", name=f"esc{h}", bufs=1)
                    nc.vector.tensor_scalar(out=esc[:], in0=esb[:], scalar1=1.0,
                                            scalar2=None, op0=OP.max)
                    ssum = sp.tile([BH, 1], f32, tag=f"ssum{h}", name=f"ssum{h}")
                    nc.vector.tensor_reduce(ssum[:], esc[:], AX.X, OP.add)
                    inv = sp.tile([BH, 1], f32, tag=f"inv{h}", name=f"inv{h}")
                    nc.vector.reciprocal(inv[:], ssum[:])
                    alph = sp.tile([BH, T], f16, tag=f"alph{h}", name=f"alph{h}", bufs=1)
                    nc.vector.tensor_scalar(out=alph[:], in0=esc[:],
                                            scalar1=inv[:, 0:1], scalar2=None,
                                            op0=OP.mult)
                    alphT = []
                    for tcn in range(2):
                        pst = pst_s[tcn]
                        nc.tensor.transpose(pst, alph[:, tcn * 128:(tcn + 1) * 128],
                                            eyeh_sb[0:BH, 0:BH])
                        at_ = sp.tile([128, BH], f16, tag=f"alphT{tcn}_{h}",
                                      name=f"alphT{tcn}_{h}")
                        nc.vector.tensor_copy(at_[:], pst)
                        alphT.append(at_)

                    # ctx.T [f, b]: rank-1 matmuls (free size 1)
                    ctxT = []
                    for fc in range(4):
                        psc = psc_s[fc]
                        for b in range(BH):
                            gb = h * BH + b
                            for tcn in range(2):
                                nc.tensor.matmul(
                                    psc[:, b:b + 1],
                                    aN_sb[(gb, tcn)][:, fc * 128:(fc + 1) * 128],
                                    alphT[tcn][:, b:b + 1],
                                    start=(tcn == 0), stop=(tcn == 1),
                                )
                        ct_ = sp.tile([128, BH], f16, tag=f"ctxT{fc}_{h}",
                                      name=f"ctxT{fc}_{h}")
                        nc.vector.tensor_copy(ct_[:], psc)
                        ctxT.append(ct_)

                    # gates in [o, b] layout; cand separate (full-scale tanh)
                    for g in range(4):
                        for oc in range(4):
                            if g == 0:
                                dst = pgc[:, oc * BH:(oc + 1) * BH]
                            else:
                                i = (g - 1) * 4 + oc
                                dst = pgs[:, i * BH:(i + 1) * BH]
                            col = g * O + oc * 128
                            nc.tensor.matmul(dst, bgr_sb[0:1, col:col + 128],
                                             ones_sb[0:1, 0:BH], start=True, stop=False)
                            for kc in range(4):
                                nc.tensor.matmul(
                                    dst, WgT_sb[kc][:, col:col + 128], sT[kc],
                                    start=False, stop=False,
                                )
                            for kc in range(4):
                                nc.tensor.matmul(
                                    dst, WgT_sb[4 + kc][:, col:col + 128], ctxT[kc][:],
                                    start=False, stop=(kc == 3),
                                )
                    cand = sp.tile([128, 4 * BH], f16, tag=f"cand{h}", name=f"cand{h}")
                    nc.scalar.activation(cand[:], pgc, AF.Tanh)
                    sig_t = sp.tile([128, 12 * BH], f16, tag=f"sigt{h}", name=f"sigt{h}")
                    nc.scalar.activation(sig_t[:], pgs, AF.Tanh)
                    sig = sp.tile([128, 12 * BH], f16, tag=f"sig{h}", name=f"sig{h}")
                    nc.gpsimd.tensor_scalar(out=sig[:], in0=sig_t[:], scalar1=0.5,
                                            scalar2=0.5, op0=OP.mult, op1=OP.add)

                    t1 = sp.tile([128, 4 * BH], f32, tag=f"t1{h}", name=f"t1{h}")
                    nc.gpsimd.tensor_tensor(out=t1[:], in0=sig[:, 0:4 * BH],
                                            in1=cand[:], op=OP.mult)
                    t2 = sp.tile([128, 4 * BH], f32, tag=f"t2{h}", name=f"t2{h}")
                    nc.gpsimd.tensor_tensor(out=t2[:], in0=sig[:, 4 * BH:8 * BH],
                                            in1=c_cur[h][:], op=OP.mult)
                    c_new = wp.tile([128, 4 * BH], f32, tag=f"c{h}", name=f"c{h}")
                    nc.gpsimd.tensor_tensor(out=c_new[:], in0=t1[:], in1=t2[:], op=OP.add)
                    tch = sp.tile([128, 4 * BH], f16, tag=f"tch{h}", name=f"tch{h}")
                    nc.scalar.activation(tch[:], c_new[:], AF.Tanh)
                    s_new = wp.tile([128, 4 * BH], f16, tag=f"s{h}", name=f"s{h}")
                    nc.gpsimd.tensor_tensor(out=s_new[:], in0=sig[:, 8 * BH:12 * BH],
                                            in1=tch[:], op=OP.mult)

                    dma(out_d[t, :, h * 4 * BH:(h + 1) * 4 * BH], s_new[:])
                    c_cur[h] = c_new
                    s_cur[h] = s_new
    nc.compile()
    return nc


def _make_runner(nc):
    """Build the sharded jit callable ONCE per module (run_bass_via_pjrt
    rebuilds it per call, costing seconds of retrace/recompile)."""
    import jax
    import numpy as _np
    from jax.sharding import Mesh, PartitionSpec
    from jax.experimental.shard_map import shard_map
    from concourse import bass2jax, mybir

    bass2jax.install_neuronx_cc_hook()
    partition_name = nc.partition_id_tensor.name if nc.partition_id_tensor else None
    in_names, out_names, out_avals, zero_outs = [], [], [], []
    for alloc in nc.m.functions[0].allocations:
        if not isinstance(alloc, mybir.MemoryLocationSet):
            continue
        name = alloc.memorylocations[0].name
        if alloc.kind == "ExternalInput":
            if name != partition_name:
                in_names.append(name)
        elif alloc.kind == "ExternalOutput":
            shape = tuple(alloc.tensor_shape)
            dtype = mybir.dt.np(alloc.dtype)
            out_names.append(name)
            out_avals.append(jax.core.ShapedArray(shape, dtype))
            zero_outs.append(_np.zeros(shape, dtype))
    n_params = len(in_names)
    n_outs = len(out_avals)
    in_names_all = list(in_names) + list(out_names)
    if partition_name is not None:
        in_names_all.append(partition_name)

    def _body(*args):
        operands = list(args)
        if partition_name is not None:
            operands.append(bass2jax.partition_id_tensor())
        outs = bass2jax._bass_exec_p.bind(
            *operands,
            out_avals=tuple(out_avals),
            in_names=tuple(in_names_all),
            out_names=tuple(out_names),
            lowering_input_output_aliases=(),
            sim_require_finite=True,
            sim_require_nnan=True,
            nc=nc,
        )
        return tuple(outs)

    donate = tuple(range(n_params, n_params + n_outs))
    devices = jax.devices()[:NCORES]
    mesh = Mesh(_np.asarray(devices), ("core",))
    sharded = jax.jit(
        shard_map(_body, mesh=mesh,
                  in_specs=(PartitionSpec("core"),) * (n_params + n_outs),
                  out_specs=(PartitionSpec("core"),) * n_outs,
                  check_rep=False),
        donate_argnums=donate, keep_unused=True,
    )

    def run(in_maps):
        concat_in = [
            np.concatenate([np.asarray(in_maps[c][nm]) for c in range(NCORES)], axis=0)
            for nm in in_names[:n_params]
        ]
        concat_zeros = [np.zeros((NCORES * z.shape[0], *z.shape[1:]), z.dtype)
                        for z in zero_outs]
        out_arrs = sharded(*concat_in, *concat_zeros)
        return [
            {nm: np.asarray(out_arrs[i]).reshape(NCORES, *out_avals[i].shape)[c]
             for i, nm in enumerate(out_names)}
            for c in range(NCORES)
        ]

    run.sharded = sharded
    run.zero_outs = zero_outs
    run.in_names = in_names[:n_params]
    run.out_names = out_names
    run.out_avals = out_avals
    return run


_BUILT = {}


def kernel(**inputs):
    a = np.asarray(inputs["a"], np.float32)
    s_prev = np.asarray(inputs["s_prev"], np.float32)
    W1 = np.asarray(inputs["W1"], np.float32)
    b1 = np.asarray(inputs["b1"], np.float32)
    W2 = np.asarray(inputs["W2"], np.float32)
    b2 = np.asarray(inputs["b2"], np.float32)
    w_c = np.asarray(inputs["w_c"], np.float32)
    w_u = np.asarray(inputs["w_u"], np.float32)
    w_f = np.asarray(inputs["w_f"], np.float32)
    w_o = np.asarray(inputs["w_o"], np.float32)
    b_c = np.asarray(inputs["b_c"], np.float32)
    b_u = np.asarray(inputs["b_u"], np.float32)
    b_f = np.asarray(inputs["b_f"], np.float32)
    b_o = np.asarray(inputs["b_o"], np.float32)
    wo = int(np.asarray(inputs["word_output"]))

    if wo not in _BUILT:
        nc_ = _build(wo)
        _BUILT[wo] = (nc_, _make_runner(nc_))
    nc, runner = _BUILT[wo]

    W1aT = np.zeros((F, MIDP), np.float16)
    W1aT[:, :MID] = W1[:, :F].T
    W1sT = np.zeros((O, MIDP), np.float16)
    W1sT[:, :MID] = W1[:, F:].T
    W2p = np.zeros((MIDP,), np.float32)
    W2p[:MID] = W2[0]
    W2c = W2p.reshape(8, 128).T.astype(np.float16)
    b1p = np.zeros((MIDP,), np.float32)
    b1p[:MID] = b1
    b1T = b1p.reshape(8, 128).T.copy()
    # gate weights: candidate full scale; u/f/o gates pre-scaled by 0.5 so
    # sigmoid(z) = 0.5*tanh(z/2)+0.5 becomes plain tanh on-device.
    WgT = np.concatenate(
        [w_c.T] + [0.5 * w.T for w in (w_u, w_f, w_o)], axis=1).astype(np.float16)
    bgr = np.concatenate(
        [b_c] + [0.5 * b for b in (b_u, b_f, b_o)]).reshape(1, 4 * O).astype(np.float16)
    common = {
        "W1aT": W1aT, "W1sT": W1sT, "W2c": W2c, "b1T": b1T,
        "b2s": np.full((BC, 1), float(b2.reshape(-1)[0]), np.float32),
        "WgT": WgT, "bgr": bgr,
        "eyeh": np.eye(16, dtype=np.float16),
        "ones16": np.ones((1, BC), np.float16),
    }
    in_maps = []
    for c in range(NCORES):
        b0 = c * BC
        ac = a[b0:b0 + BC]
        # aT t-major: aT[f, t*BC + b] = a[b, t, f]
        aT = np.ascontiguousarray(ac.transpose(2, 1, 0).reshape(F, BT)).astype(np.float16)
        # sP [p, (h,oc,bh)]: sP[p, h*32+oc*8+bh] = s_prev[b0 + h*8+bh, oc*128+p]
        sP = np.ascontiguousarray(
            np.transpose(s_prev[b0:b0 + BC].T.reshape(4, 128, 2, BH), (1, 2, 0, 3))
        ).reshape(128, 4 * BC).astype(np.float16)
        in_maps.append({
            **common,
            "aT": aT,
            "aN": np.ascontiguousarray(ac.reshape(BT, F)).astype(np.float16),
            "sP": sP,
        })

    results = None
    for attempt in range(4):
        try:
            results = runner(in_maps)
            break
        except Exception:
            if attempt == 3:
                raise
            import time as _time
            _time.sleep(1.0)
            if attempt >= 1:
                runner = _make_runner(nc)
                _BUILT[wo] = (nc, runner)
    out = np.empty((B, wo, O), np.float32)
    for c in range(NCORES):
        # raw [wo, 128, (h,oc,bh)] fp16 -> [b = h*8+bh, wo, o = oc*128+p]
        raw = results[c]["out"].reshape(wo, 128, 2, 4, BH)
        out[c * BC:(c + 1) * BC] = np.ascontiguousarray(
            np.transpose(raw, (2, 4, 0, 3, 1))).reshape(BC, wo, O).astype(np.float32)
    return out
